# revision 7
# baseline (speedup 1.0000x reference)
"""Trainium2 Bass kernel for nn_K_ANP_41188736369107.

Math: the reference computes
    std = std(x, axis=-1, ddof=1); p = 2 + log1p(mean(std))
    norm = (sum |x|^p)^(1/p); lc = norm/(norm+eps); e = exp(lc)
    out = mean(x*e, -1) / mean(broadcast(e), -1)
Since e is constant along the reduced axis, up/down == mean(x, axis=-1)
exactly (the std/p/norm/exp factors cancel); verified numerically at
~2.6e-7 norm relative error in fp32.  So the kernel is a row-mean over
the last axis (K=64), data-parallel over the batch axis across 8 cores.

Layout per core: x[i] (256,512,64) viewed as [T, 128, L]: tile t,
partition p holds L/64 consecutive rows of K=64.  Raw-Bass pipeline
(not Tile: Tile embeds two sync-waits into slot-reusing DMAs, which
walrus rejects for DMA_DIRECT2D descriptors):

  SP   : in-DMA tile t   (waits DVE released slot t-NBUF via wait_ge)
  DVE  : row-sum reduce  (waits in-DMA done, ACT released st slot)
  ACT  : scale by 1/K, then fires the out-DMA from its own HWDGE queue

All waits are standalone sequencer instructions, so every DMACopy
carries zero embedded waits.  The DMA stream (~33.5 MB/core at the
358 GB/s HBM roofline, ~100 us) hides the DVE reduce (~70 us).
"""

from contextlib import ExitStack

import numpy as np

import concourse.bass as bass
import concourse.mybir as mybir
from concourse.bass_utils import run_bass_kernel_spmd

K = 64          # reduced (neighbor) axis
P = 128         # SBUF partitions
N_CORES = 8
B, C, G = 8, 256, 512   # knn_x_w shape is (B, C, G, K)

# per-core tiling: C*G*K = 8,388,608 f32 = T * P * L
T = 16          # tiles per core
L = (C * G * K) // (T * P)   # f32 per partition line
R = L // K      # rows of K per partition line
NBUF = 8        # input-tile ring slots
OBUF = 4        # sum/out-tile ring slots

F32 = mybir.dt.float32


def build_nc(t_tiles=T, line=L, nbuf=NBUF, obuf=OBUF):
    rows = line // K
    nc = bass.Bass()
    x = nc.dram_tensor("x", [t_tiles, P, line], F32, kind="ExternalInput")
    y = nc.dram_tensor("y", [t_tiles, P, rows], F32, kind="ExternalOutput")

    with ExitStack() as ctx:
        xbufs = [
            ctx.enter_context(nc.sbuf_tensor(f"xb{i}", [P, line], F32))
            for i in range(nbuf)
        ]
        sbufs = [
            ctx.enter_context(nc.sbuf_tensor(f"sb{i}", [P, rows], F32))
            for i in range(obuf)
        ]
        ybufs = [
            ctx.enter_context(nc.sbuf_tensor(f"yb{i}", [P, rows], F32))
            for i in range(obuf)
        ]
        # One DMA-completion semaphore per ring slot: a single DMA's 16
        # increments come from 16 independent SDMA engines, so a shared
        # counter can hit 16*(t+1) from a MIX of two in-flight DMAs.
        # Per-slot counters are monotone (same-slot DMAs serialize on the
        # slot-free wait), so >= 16*uses is race-free.
        in_sems = [
            ctx.enter_context(nc.semaphore(f"in_sem{i}")) for i in range(nbuf)
        ]
        out_sems = [
            ctx.enter_context(nc.semaphore(f"out_sem{i}")) for i in range(obuf)
        ]
        dve_sem = ctx.enter_context(nc.semaphore("dve_sem"))
        mul_sem = ctx.enter_context(nc.semaphore("mul_sem"))

        with nc.Block() as block:

            @block.sync
            def _(sp):
                for t in range(t_tiles):
                    if t >= nbuf:
                        # slot free once DVE consumed tile t-nbuf (which
                        # transitively implies that tile's DMA completed)
                        sp.wait_ge(dve_sem, t - nbuf + 1)
                    sp.dma_start(xbufs[t % nbuf][:], x[t]).then_inc(
                        in_sems[t % nbuf], 16
                    )

            @block.vector
            def _(v):
                for t in range(t_tiles):
                    v.wait_ge(in_sems[t % nbuf], 16 * (t // nbuf + 1))
                    if t >= obuf:
                        v.wait_ge(mul_sem, t - obuf + 1)
                    view = xbufs[t % nbuf][:].rearrange("p (r k) -> p r k", k=K)
                    v.reduce_sum(
                        sbufs[t % obuf][:], view, axis=mybir.AxisListType.X
                    ).then_inc(dve_sem, 1)

            @block.scalar
            def _(act):
                for t in range(t_tiles):
                    act.wait_ge(dve_sem, t + 1)
                    if t >= obuf:
                        # yt slot free once its previous out-DMA completed
                        act.wait_ge(out_sems[t % obuf], 16 * (t // obuf))
                    act.mul(ybufs[t % obuf][:], sbufs[t % obuf][:], 1.0 / K).then_inc(
                        mul_sem, 1
                    )
                    # seq-side wait for our own mul before triggering the DMA
                    act.wait_ge(mul_sem, t + 1)
                    act.dma_start(y[t], ybufs[t % obuf][:]).then_inc(
                        out_sems[t % obuf], 16
                    )

    return nc


_NC_CACHE = {}


def _get_nc():
    if "nc" not in _NC_CACHE:
        _NC_CACHE["nc"] = build_nc()
    return _NC_CACHE["nc"]


def _run(x, trace=False, tmpdir=None):
    """x: (B, C, G, K) float32 -> (B, C, G) float32.  Returns (out, results)."""
    x = np.ascontiguousarray(x, dtype=np.float32)
    assert x.shape == (B, C, G, K), x.shape
    nc = _get_nc()
    in_maps = [{"x": x[i].reshape(T, P, L)} for i in range(N_CORES)]
    res = run_bass_kernel_spmd(
        nc, in_maps, core_ids=list(range(N_CORES)), trace=trace, tmpdir=tmpdir
    )
    out = np.stack(
        [res.results[i]["y"].reshape(C, G) for i in range(N_CORES)], axis=0
    )
    return out, res


def kernel(**inputs):
    out, _ = _run(inputs["knn_x_w"])
    return out


# revision 8
# speedup vs baseline: 1.1763x; 1.1763x over previous
"""Trainium2 Bass kernel for nn_K_ANP_41188736369107.

Math: the reference computes
    std = std(x, axis=-1, ddof=1); p = 2 + log1p(mean(std))
    norm = (sum |x|^p)^(1/p); lc = norm/(norm+eps); e = exp(lc)
    out = mean(x*e, -1) / mean(broadcast(e), -1)
Since e is constant along the reduced axis, up/down == mean(x, axis=-1)
exactly (the std/p/norm/exp factors cancel); verified numerically at
~2.6e-7 norm relative error in fp32.  So the kernel is a row-mean over
the last axis (K=64), data-parallel over the batch axis across 8 cores.

Per-core: x[i] (256,512,64) flattened; tile g covers 128 partition
lines of `line` consecutive f32 (line/64 rows of K=64 per partition).
Raw-Bass pipeline (not Tile: Tile embeds two sync-waits into
slot-reusing DMAs, which walrus rejects on DMA_DIRECT2D descriptors):

  SP   : in-DMA tile g   (slot-free wait via standalone wait_ge)
  DVE  : row-sum reduce
  ACT  : scale by 1/K, then fires the out-DMA from its own HWDGE queue

The schedule uses small tiles at the head (short pipeline-fill
latency) and tail (short final reduce + DMA receipt), big tiles in the
steady state where the 358 GB/s HBM stream is the only limit.
Per-ring-slot DMA semaphores: one DMA's 16 increments come from 16
independent SDMA engines, so a shared counter would be racy.
"""

from contextlib import ExitStack

import numpy as np

import concourse.bass as bass
import concourse.mybir as mybir
from concourse.bass_utils import run_bass_kernel_spmd

K = 64          # reduced (neighbor) axis
P = 128         # SBUF partitions
N_CORES = 8
B, C, G = 8, 256, 512   # knn_x_w shape is (B, C, G, K)

ELEMS = C * G * K            # 8,388,608 f32 per core
LINES = ELEMS // P           # 65,536 f32 per partition

SMALL = 1024                 # small-tile line length (512 KiB tile)
BIG = 4096                   # big-tile line length  (2 MiB tile)
N_HEAD = 4                   # small tiles at the head
N_TAIL = 4                   # small tiles at the tail
N_BIG = (LINES - (N_HEAD + N_TAIL) * SMALL) // BIG   # 14
SCHEDULE = [SMALL] * N_HEAD + [BIG] * N_BIG + [SMALL] * N_TAIL
assert sum(SCHEDULE) == LINES

NBIGBUF = 8     # big-tile ring slots (8 * 16 KiB/partition)
OBUF = 6        # sum/out-tile ring slots

F32 = mybir.dt.float32


def build_nc(schedule=None, nbigbuf=NBIGBUF, obuf=OBUF):
    schedule = list(SCHEDULE if schedule is None else schedule)
    lines = sum(schedule)
    n_small = sum(1 for s in schedule if s != BIG)
    rows_max = max(schedule) // K

    nc = bass.Bass()
    x = nc.dram_tensor("x", [P * lines], F32, kind="ExternalInput")
    y = nc.dram_tensor("y", [2 * lines], F32, kind="ExternalOutput")

    with ExitStack() as ctx:
        small_bufs = [
            ctx.enter_context(nc.sbuf_tensor(f"smb{i}", [P, SMALL], F32))
            for i in range(n_small)
        ]
        big_bufs = [
            ctx.enter_context(nc.sbuf_tensor(f"bgb{i}", [P, BIG], F32))
            for i in range(nbigbuf)
        ]
        sbufs = [
            ctx.enter_context(nc.sbuf_tensor(f"sb{i}", [P, rows_max], F32))
            for i in range(obuf)
        ]
        ybufs = [
            ctx.enter_context(nc.sbuf_tensor(f"yb{i}", [P, rows_max], F32))
            for i in range(obuf)
        ]
        sm_sems = [
            ctx.enter_context(nc.semaphore(f"sm_sem{i}")) for i in range(n_small)
        ]
        bg_sems = [
            ctx.enter_context(nc.semaphore(f"bg_sem{i}")) for i in range(nbigbuf)
        ]
        out_sems = [
            ctx.enter_context(nc.semaphore(f"out_sem{i}")) for i in range(obuf)
        ]
        dve_sem = ctx.enter_context(nc.semaphore("dve_sem"))
        mul_sem = ctx.enter_context(nc.semaphore("mul_sem"))

        # static per-tile metadata
        tiles = []  # (line, buf_ap, in_sem, wait_val_for_dve, slot_free_dve_wait)
        prefix = 0
        si = 0
        big_uses = [0] * nbigbuf
        bi = 0
        for g, line in enumerate(schedule):
            if line != BIG:
                buf, sem, uses = small_bufs[si], sm_sems[si], 1
                slot_prev_tile = None
                si += 1
            else:
                s = bi % nbigbuf
                buf, sem = big_bufs[s], bg_sems[s]
                big_uses[s] += 1
                uses = big_uses[s]
                # global index of the tile that previously held this slot
                slot_prev_tile = g - nbigbuf if bi >= nbigbuf else None
                bi += 1
            tiles.append(
                dict(
                    g=g,
                    line=line,
                    rows=line // K,
                    prefix=prefix,
                    buf=buf,
                    sem=sem,
                    uses=uses,
                    slot_prev=slot_prev_tile,
                )
            )
            prefix += line

        with nc.Block(no_gpsimd_drain=True) as block:

            @block.sync
            def _(sp):
                for tl in tiles:
                    if tl["slot_prev"] is not None:
                        # slot free once DVE consumed its previous occupant
                        # (transitively implies that tile's DMA completed)
                        sp.wait_ge(dve_sem, tl["slot_prev"] + 1)
                    src = x[
                        P * tl["prefix"] : P * (tl["prefix"] + tl["line"])
                    ].rearrange("(p l) -> p l", l=tl["line"])
                    sp.dma_start(tl["buf"][:], src).then_inc(tl["sem"], 16)

            @block.vector
            def _(v):
                for tl in tiles:
                    g = tl["g"]
                    v.wait_ge(tl["sem"], 16 * tl["uses"])
                    if g >= obuf:
                        v.wait_ge(mul_sem, g - obuf + 1)
                    view = tl["buf"][:].rearrange("p (r k) -> p r k", k=K)
                    v.reduce_sum(
                        sbufs[g % obuf][:, : tl["rows"]],
                        view,
                        axis=mybir.AxisListType.X,
                    ).then_inc(dve_sem, 1)

            @block.scalar
            def _(act):
                for tl in tiles:
                    g = tl["g"]
                    o = g % obuf
                    act.wait_ge(dve_sem, g + 1)
                    if g >= obuf:
                        # yt slot free once its previous out-DMA completed
                        act.wait_ge(out_sems[o], 16 * (g // obuf))
                    act.mul(
                        ybufs[o][:, : tl["rows"]],
                        sbufs[o][:, : tl["rows"]],
                        1.0 / K,
                    ).then_inc(mul_sem, 1)
                    # seq-side wait for our own mul before triggering the DMA
                    act.wait_ge(mul_sem, g + 1)
                    dst = y[
                        2 * tl["prefix"] : 2 * tl["prefix"] + P * tl["rows"]
                    ].rearrange("(p r) -> p r", r=tl["rows"])
                    act.dma_start(dst, ybufs[o][:, : tl["rows"]]).then_inc(
                        out_sems[o], 16
                    )

    return nc


_NC_CACHE = {}


def _get_nc():
    if "nc" not in _NC_CACHE:
        _NC_CACHE["nc"] = build_nc()
    return _NC_CACHE["nc"]


def _run(x, trace=False, tmpdir=None):
    """x: (B, C, G, K) float32 -> (B, C, G) float32.  Returns (out, results)."""
    x = np.ascontiguousarray(x, dtype=np.float32)
    assert x.shape == (B, C, G, K), x.shape
    nc = _get_nc()
    in_maps = [{"x": x[i].reshape(-1)} for i in range(N_CORES)]
    res = run_bass_kernel_spmd(
        nc, in_maps, core_ids=list(range(N_CORES)), trace=trace, tmpdir=tmpdir
    )
    out = np.stack(
        [res.results[i]["y"].reshape(C, G) for i in range(N_CORES)], axis=0
    )
    return out, res


def kernel(**inputs):
    out, _ = _run(inputs["knn_x_w"])
    return out


# revision 10
# speedup vs baseline: 1.1954x; 1.0162x over previous
"""Trainium2 Bass kernel for nn_K_ANP_41188736369107.

Math: the reference computes
    std = std(x, axis=-1, ddof=1); p = 2 + log1p(mean(std))
    norm = (sum |x|^p)^(1/p); lc = norm/(norm+eps); e = exp(lc)
    out = mean(x*e, -1) / mean(broadcast(e), -1)
Since e is constant along the reduced axis, up/down == mean(x, axis=-1)
exactly (the std/p/norm/exp factors cancel); verified numerically at
~2.6e-7 norm relative error in fp32.  So the kernel is a row-mean over
the last axis (K=64), data-parallel over the batch axis across 8 cores.

Per-core: x[i] (256,512,64) flattened; tile g covers 128 partition
lines of `line` consecutive f32 (line/64 rows of K=64 per partition).
Raw-Bass pipeline (not Tile: Tile embeds two sync-waits into
slot-reusing DMAs, which walrus rejects on DMA_DIRECT2D descriptors):

  SP   : in-DMA tile g   (slot-free wait via standalone wait_ge)
  DVE  : row-sum reduce
  ACT  : scale by 1/K, then fires the out-DMA from its own HWDGE queue

The schedule uses small tiles at the head (short pipeline-fill
latency) and tail (short final reduce + DMA receipt), big tiles in the
steady state where the 358 GB/s HBM stream is the only limit.
Per-ring-slot DMA semaphores: one DMA's 16 increments come from 16
independent SDMA engines, so a shared counter would be racy.
"""

from contextlib import ExitStack

import numpy as np

import concourse.bass as bass
import concourse.mybir as mybir
from concourse.bass_utils import run_bass_kernel_spmd

K = 64          # reduced (neighbor) axis
P = 128         # SBUF partitions
N_CORES = 8
B, C, G = 8, 256, 512   # knn_x_w shape is (B, C, G, K)

ELEMS = C * G * K            # 8,388,608 f32 per core
LINES = ELEMS // P           # 65,536 f32 per partition

SMALL = 1024                 # small-tile line length (512 KiB tile)
BIG = 2048                   # big-tile line length  (1 MiB tile)
N_HEAD = 4                   # small tiles at the head
N_TAIL = 4                   # small tiles at the tail
N_BIG = (LINES - (N_HEAD + N_TAIL) * SMALL) // BIG   # 28
SCHEDULE = [SMALL] * N_HEAD + [BIG] * N_BIG + [SMALL] * N_TAIL
assert sum(SCHEDULE) == LINES

NBIGBUF = 16    # big-tile ring slots (16 * 8 KiB/partition)
OBUF = 6        # sum/out-tile ring slots

F32 = mybir.dt.float32


def build_nc(schedule=None, nbigbuf=NBIGBUF, obuf=OBUF):
    schedule = list(SCHEDULE if schedule is None else schedule)
    lines = sum(schedule)
    n_small = sum(1 for s in schedule if s != BIG)
    rows_max = max(schedule) // K

    # Bass.__init__ emits four const memsets plus an all-engine barrier
    # (~3.5us EVSEM butterfly + drain) before any user code.  Nothing in
    # this kernel reads those consts, so skip the init barrier; the
    # Block-exit barrier (needed for completion) is emitted after the
    # patch is restored.
    _orig_barrier = bass.Bass.all_engine_barrier
    bass.Bass.all_engine_barrier = lambda self, *a, **k: None
    try:
        nc = bass.Bass()
    finally:
        bass.Bass.all_engine_barrier = _orig_barrier
    x = nc.dram_tensor("x", [P * lines], F32, kind="ExternalInput")
    y = nc.dram_tensor("y", [2 * lines], F32, kind="ExternalOutput")

    with ExitStack() as ctx:
        small_bufs = [
            ctx.enter_context(nc.sbuf_tensor(f"smb{i}", [P, SMALL], F32))
            for i in range(n_small)
        ]
        big_bufs = [
            ctx.enter_context(nc.sbuf_tensor(f"bgb{i}", [P, BIG], F32))
            for i in range(nbigbuf)
        ]
        sbufs = [
            ctx.enter_context(nc.sbuf_tensor(f"sb{i}", [P, rows_max], F32))
            for i in range(obuf)
        ]
        ybufs = [
            ctx.enter_context(nc.sbuf_tensor(f"yb{i}", [P, rows_max], F32))
            for i in range(obuf)
        ]
        sm_sems = [
            ctx.enter_context(nc.semaphore(f"sm_sem{i}")) for i in range(n_small)
        ]
        bg_sems = [
            ctx.enter_context(nc.semaphore(f"bg_sem{i}")) for i in range(nbigbuf)
        ]
        out_sems = [
            ctx.enter_context(nc.semaphore(f"out_sem{i}")) for i in range(obuf)
        ]
        dve_sem = ctx.enter_context(nc.semaphore("dve_sem"))
        mul_sem = ctx.enter_context(nc.semaphore("mul_sem"))

        # static per-tile metadata
        tiles = []  # (line, buf_ap, in_sem, wait_val_for_dve, slot_free_dve_wait)
        prefix = 0
        si = 0
        big_uses = [0] * nbigbuf
        bi = 0
        for g, line in enumerate(schedule):
            if line != BIG:
                buf, sem, uses = small_bufs[si], sm_sems[si], 1
                slot_prev_tile = None
                si += 1
            else:
                s = bi % nbigbuf
                buf, sem = big_bufs[s], bg_sems[s]
                big_uses[s] += 1
                uses = big_uses[s]
                # global index of the tile that previously held this slot
                slot_prev_tile = g - nbigbuf if bi >= nbigbuf else None
                bi += 1
            tiles.append(
                dict(
                    g=g,
                    line=line,
                    rows=line // K,
                    prefix=prefix,
                    buf=buf,
                    sem=sem,
                    uses=uses,
                    slot_prev=slot_prev_tile,
                )
            )
            prefix += line

        with nc.Block(no_gpsimd_drain=True) as block:

            @block.sync
            def _(sp):
                for tl in tiles:
                    if tl["slot_prev"] is not None:
                        # slot free once DVE consumed its previous occupant
                        # (transitively implies that tile's DMA completed)
                        sp.wait_ge(dve_sem, tl["slot_prev"] + 1)
                    src = x[
                        P * tl["prefix"] : P * (tl["prefix"] + tl["line"])
                    ].rearrange("(p l) -> p l", l=tl["line"])
                    sp.dma_start(tl["buf"][:], src).then_inc(tl["sem"], 16)

            @block.vector
            def _(v):
                for tl in tiles:
                    g = tl["g"]
                    v.wait_ge(tl["sem"], 16 * tl["uses"])
                    if g >= obuf:
                        v.wait_ge(mul_sem, g - obuf + 1)
                    view = tl["buf"][:].rearrange("p (r k) -> p r k", k=K)
                    v.reduce_sum(
                        sbufs[g % obuf][:, : tl["rows"]],
                        view,
                        axis=mybir.AxisListType.X,
                    ).then_inc(dve_sem, 1)

            @block.scalar
            def _(act):
                for tl in tiles:
                    g = tl["g"]
                    o = g % obuf
                    act.wait_ge(dve_sem, g + 1)
                    if g >= obuf:
                        # yt slot free once its previous out-DMA completed
                        act.wait_ge(out_sems[o], 16 * (g // obuf))
                    act.mul(
                        ybufs[o][:, : tl["rows"]],
                        sbufs[o][:, : tl["rows"]],
                        1.0 / K,
                    ).then_inc(mul_sem, 1)
                    # seq-side wait for our own mul before triggering the DMA
                    act.wait_ge(mul_sem, g + 1)
                    dst = y[
                        2 * tl["prefix"] : 2 * tl["prefix"] + P * tl["rows"]
                    ].rearrange("(p r) -> p r", r=tl["rows"])
                    act.dma_start(dst, ybufs[o][:, : tl["rows"]]).then_inc(
                        out_sems[o], 16
                    )

    return nc


_NC_CACHE = {}


def _get_nc():
    if "nc" not in _NC_CACHE:
        _NC_CACHE["nc"] = build_nc()
    return _NC_CACHE["nc"]


def _run(x, trace=False, tmpdir=None):
    """x: (B, C, G, K) float32 -> (B, C, G) float32.  Returns (out, results)."""
    x = np.ascontiguousarray(x, dtype=np.float32)
    assert x.shape == (B, C, G, K), x.shape
    nc = _get_nc()
    in_maps = [{"x": x[i].reshape(-1)} for i in range(N_CORES)]
    res = run_bass_kernel_spmd(
        nc, in_maps, core_ids=list(range(N_CORES)), trace=trace, tmpdir=tmpdir
    )
    out = np.stack(
        [res.results[i]["y"].reshape(C, G) for i in range(N_CORES)], axis=0
    )
    return out, res


def kernel(**inputs):
    out, _ = _run(inputs["knn_x_w"])
    return out
